# revision 2
# baseline (speedup 1.0000x reference)
"""4-layer GraphSAGE (mean aggr) on 8 TRN2 NeuronCores — gather + S-matmul.

Strategy (dst-owner node partitioning, NO scatter-add):
  - Nodes partitioned across 8 cores by dst ownership (12500 each, padded to
    12544 = 98*128).  Feature tables (bf16, node-major) are replicated via
    chunked AllGather (7 chunks of 14 blocks) so AG overlaps compute.
  - Aggregation per 128-node block via TensorE "segment matmul": for each
    128-edge chunk, PSUM[f, n] += G_chunk.T @ S_chunk where G = gathered src
    features [128e, 128f] (bf16) and S[e, n] = invc[dst] * onehot(dst%128)
    (bf16, folds the 1/deg mean normalization).  Pad edges have all-zero S
    rows, so no zeroing passes and no scatter WAW chains exist at all.
  - S is generated once on-device by gathering rows of a small uploaded
    [12544, 128] scaled-onehot table with idx = dst slot, teed to DRAM and
    re-streamed (contiguous reads) for layers 2-4.
  - int16 gather indices: table stored z-chunk-major ([zchunk, core, 1792,
    128]) so each AllGather chunk is contiguous; gathers are split over 4
    row ranges of <=28672 rows so indices fit in int16.
  - Edge slots per (block, range) are padded to a multiple of 128 and to the
    max over the 8 cores (SPMD: all cores run the same program structure,
    computed from the actual graph at first call).
  - Phase C per block: ACT copies PSUM agg -> SBUF, weight matmuls
    (agg @ Wl + h @ Wr, fp32), Relu+bias on ACT, bf16 cast + PE transpose to
    node-major for the next table, fp32 feature-major copy kept in DRAM for
    the next layer's self term.  Layer 1 is transform-first (table1 = x@Wl1)
    and adds Wr1 @ x directly into the agg PSUM.
"""

import numpy as np

# ---------------------------------------------------------------- constants
NCORES = 8
N = 100000
E = 1600000
F_IN = 16
H = 128
SHARD = 12500
NBLK = 98
SHARD_P = NBLK * 128          # 12544
GRP = 14                      # blocks per group == AllGather chunk
NGRP = 7
ZROWS = GRP * 128             # 1792 rows per (core, zchunk)
ZTBL = NCORES * ZROWS         # 14336 table rows per zchunk
TBL_ROWS = NGRP * ZTBL        # 100352
NR = 4                        # gather ranges (int16: 25088 rows each)
RANGE_ROWS = TBL_ROWS // NR   # 25088 = 2 core-shards
MAXC = 8                      # max 128-edge chunks per gather instruction
PAD_SLOT = SHARD_P - 1        # dst slot for pad edges: oh row is all-zero

_compiled = None
_compiled_key = None


# ---------------------------------------------------------------- layout
def _layout(CH):
    """Static program structure from CH[98][4] chunk counts.

    Chunk order: for zg, for g, for b in group(zg): CH[b][g] chunks.
    Instructions split each (zg, g) span into ceil(n/MAXC) near-equal parts.
    Returns dict with chunk_base[98][4], instrs[zg][g] = list of
    (chunk0, nch), chunkinfo[totch] = (block, start, stop), totch.
    """
    CH = np.asarray(CH, dtype=np.int64)
    assert CH.shape == (NBLK, NR) and (CH >= 1).all()
    chunk_base = np.zeros((NBLK, NR), dtype=np.int64)
    instrs = [[[] for _ in range(NR)] for _ in range(NGRP)]
    chunkinfo = []
    tot = 0
    for zg in range(NGRP):
        blocks = range(zg * GRP, (zg + 1) * GRP)
        for g in range(NR):
            span0 = tot
            for b in blocks:
                chunk_base[b, g] = tot
                # PSUM accumulation-group flags are per 4-block PSUM TILE
                # (start clears whole-bank state, so exactly one start/stop
                # per tile): tile jj covers blocks [t0, t1].
                jj = (b - zg * GRP) // 4
                t0 = zg * GRP + jj * 4
                t1 = min(t0 + 3, (zg + 1) * GRP - 1)
                for k in range(CH[b, g]):
                    first = g == 0 and k == 0 and b == t0
                    last = g == NR - 1 and k == CH[b, g] - 1 and b == t1
                    chunkinfo.append((b, first, last))
                tot += CH[b, g]
            n = tot - span0
            nin = -(-n // MAXC)
            per = -(-n // nin)
            c0 = span0
            while c0 < tot:
                nch = min(per, tot - c0)
                instrs[zg][g].append((c0, nch))
                c0 += nch
    return {
        "chunk_base": chunk_base,
        "instrs": instrs,
        "chunkinfo": chunkinfo,
        "totch": tot,
    }


def _compute_ch(edge_index):
    src = np.asarray(edge_index[0]).astype(np.int64)
    dst = np.asarray(edge_index[1]).astype(np.int64)
    core = dst // SHARD
    dst_loc = dst - core * SHARD
    b = dst_loc // 128
    cs = src // SHARD
    g = cs // 2
    counts = np.zeros((NCORES, NBLK, NR), dtype=np.int64)
    np.add.at(counts, (core, b, g), 1)
    ch = -(-counts.max(axis=0) // 128)
    return np.maximum(ch, 1)


# ---------------------------------------------------------------- program
def _build_program(CH):
    import concourse.bacc as bacc
    import concourse.masks as masks
    import concourse.mybir as mybir
    import concourse.tile as tile

    fp32 = mybir.dt.float32
    bf16 = mybir.dt.bfloat16
    i16 = mybir.dt.int16
    AF = mybir.ActivationFunctionType

    lay = _layout(CH)
    TOTCH = lay["totch"]
    IDXCOLS = TOTCH * 8
    chunkinfo = lay["chunkinfo"]
    instrs = lay["instrs"]

    nc = bacc.Bacc(
        "TRN2",
        target_bir_lowering=False,
        debug=False,
        enable_asserts=False,
        num_devices=NCORES,
    )

    # -------- I/O
    xt_d = nc.dram_tensor("xt", [F_IN, SHARD_P], fp32, kind="ExternalInput")
    gidx_d = nc.dram_tensor("gidx", [128, IDXCOLS], i16, kind="ExternalInput")
    sdata_d = nc.dram_tensor("sdata", [128, TOTCH * 128], bf16,
                             kind="ExternalInput")
    w_d = {}
    for l in range(1, 5):
        din = F_IN if l == 1 else H
        w_d[f"wl{l}"] = nc.dram_tensor(f"wl{l}", [din, H], fp32, kind="ExternalInput")
        w_d[f"wr{l}"] = nc.dram_tensor(f"wr{l}", [din, H], fp32, kind="ExternalInput")
        w_d[f"b{l}"] = nc.dram_tensor(f"b{l}", [128, 1], fp32, kind="ExternalInput")
    out_d = nc.dram_tensor("out", [128, SHARD_P], fp32, kind="ExternalOutput")

    with tile.TileContext(nc) as tc:
        with (
            tc.tile_pool(name="dram", bufs=1, space="DRAM") as dpool,
            tc.tile_pool(name="const", bufs=1) as cpool,
            tc.tile_pool(name="ix", bufs=6) as xpool,
            tc.tile_pool(name="gat", bufs=3) as gpool,
            tc.tile_pool(name="smat", bufs=3) as spool,
            tc.tile_pool(name="hin", bufs=2) as hpool,
            tc.tile_pool(name="xtin", bufs=2) as xtpool,
            tc.tile_pool(name="work", bufs=2) as wpool,
            tc.tile_pool(name="psA", bufs=5, space="PSUM") as papool,
            tc.tile_pool(name="ps2", bufs=2, space="PSUM") as p2pool,
            tc.tile_pool(name="pT", bufs=1, space="PSUM") as ptpool,
        ):
            tbl = [
                dpool.tile([TBL_ROWS, H], bf16, addr_space="Shared",
                           name=f"tbl{l}")
                for l in range(4)
            ]
            sh = [dpool.tile([SHARD_P, H], bf16, name=f"sh{l}") for l in range(4)]
            hfm = [
                dpool.tile([128, SHARD_P], fp32, name=f"hfm{l}")
                for l in range(1, 4)
            ]

            # -------- constants
            ident = cpool.tile([128, 128], fp32, name="ident")
            masks.make_identity(nc, ident[:])
            w_sb = {}
            for l in range(1, 5):
                din = F_IN if l == 1 else H
                for nm in (f"wl{l}", f"wr{l}"):
                    t = cpool.tile([din, H], fp32, name=f"{nm}_sb")
                    nc.sync.dma_start(t[:], w_d[nm].ap())
                    w_sb[nm] = t
                t = cpool.tile([128, 1], fp32, name=f"b{l}_sb")
                nc.sync.dma_start(t[:], w_d[f"b{l}"].ap())
                w_sb[f"b{l}"] = t

            def nm_view(ap):
                return ap.rearrange("(b p) f -> p b f", p=128)

            def allgather(l):
                nc.gpsimd.collective_compute(
                    "AllGather",
                    mybir.AluOpType.bypass,
                    replica_groups=[list(range(NCORES))],
                    ins=[sh[l][:, :].opt()],
                    outs=[tbl[l][:, :].opt()],
                )

            # ---- phase A: sh0 = bf16(x @ Wl1) node-major; AG -> tbl0
            for zg in range(NGRP):
                xt_sb = xtpool.tile([F_IN, ZROWS], fp32, tag="xtA")
                nc.sync.dma_start(
                    xt_sb[:], xt_d.ap()[:, zg * ZROWS:(zg + 1) * ZROWS]
                )
                nmst = wpool.tile([128, GRP, 128], bf16, tag="nmA")
                for q in range(0, GRP, 4):
                    nb = min(4, GRP - q)
                    ps = p2pool.tile([128, 4, 128], fp32, tag="ps2")
                    psf = ps.rearrange("p a b -> p (a b)")
                    nc.tensor.matmul(
                        psf[:, :nb * 128], w_sb["wl1"][:],
                        xt_sb[:, q * 128:(q + nb) * 128],
                        start=True, stop=True,
                    )
                    csb = wpool.tile([128, 4, 128], fp32, tag="csbA", bufs=2)
                    nc.scalar.copy(
                        csb.rearrange("p a b -> p (a b)")[:, :nb * 128],
                        psf[:, :nb * 128],
                    )
                    pt = ptpool.tile([128, 4, 128], fp32, tag="pT")
                    for j in range(nb):
                        nc.tensor.transpose(pt[:, j, :], csb[:, j, :], ident[:])
                    nc.vector.tensor_copy(nmst[:, q:q + nb, :], pt[:, :nb, :])
                nc.sync.dma_start(
                    nm_view(sh[0])[:, zg * GRP:(zg + 1) * GRP, :], nmst[:]
                )
            allgather(0)

            # ---- layers
            for l in range(1, 5):
                for zg in range(NGRP):
                    # phase B: gathers + segment matmuls
                    ps_of = {}
                    for jj in range(4):
                        nb = min(4, GRP - jj * 4)
                        t = papool.tile([128, 4, 128], fp32, tag="psA")
                        for j in range(nb):
                            ps_of[zg * GRP + jj * 4 + j] = t[:, j, :]
                    for g in range(NR):
                        for (c0, nch) in instrs[zg][g]:
                            nidx = nch * 128
                            ix = xpool.tile([128, MAXC * 8], i16, tag="gix")
                            nc.sync.dma_start(
                                ix[:, :nch * 8],
                                gidx_d.ap()[:, c0 * 8:(c0 + nch) * 8],
                            )
                            gt = gpool.tile([128, MAXC, 128], bf16, tag="gt")
                            nc.gpsimd.dma_gather(
                                gt[:, :nch, :],
                                tbl[l - 1][g * RANGE_ROWS:
                                           (g + 1) * RANGE_ROWS, :],
                                ix[:, :nch * 8],
                                num_idxs=nidx, num_idxs_reg=nidx,
                                elem_size=H,
                            )
                            ss = spool.tile([128, MAXC, 128], bf16, tag="ss")
                            nc.sync.dma_start(
                                ss.rearrange("p a b -> p (a b)")[:, :nch * 128],
                                sdata_d.ap()[:, c0 * 128:(c0 + nch) * 128],
                            )
                            for c in range(nch):
                                b, first, last = chunkinfo[c0 + c]
                                stop = last and l != 1
                                nc.tensor.matmul(
                                    ps_of[b], gt[:, c, :], ss[:, c, :],
                                    start=first, stop=stop,
                                )

                    # phase C
                    if l == 1:
                        xt_sb = xtpool.tile([F_IN, ZROWS], fp32, tag="xtC")
                        nc.sync.dma_start(
                            xt_sb[:], xt_d.ap()[:, zg * ZROWS:(zg + 1) * ZROWS]
                        )
                    elif l >= 2:
                        hin = hpool.tile([128, GRP, 128], fp32, tag="hin")
                        nc.sync.dma_start(
                            hin.rearrange("p a b -> p (a b)"),
                            hfm[l - 2][:, zg * ZROWS:(zg + 1) * ZROWS],
                        )
                    if l < 4:
                        hst = wpool.tile([128, GRP, 128], fp32, tag="hst")
                        nmst = wpool.tile([128, GRP, 128], bf16, tag="nmC")
                    else:
                        ost = wpool.tile([128, GRP, 128], fp32, tag="ost")
                    func = AF.Relu if l < 4 else AF.Identity
                    for jj in range(4):
                        nb = min(4, GRP - jj * 4)
                        b0 = zg * GRP + jj * 4
                        paf = None
                        if l == 1:
                            for j in range(nb):
                                nc.tensor.matmul(
                                    ps_of[b0 + j], w_sb["wr1"][:],
                                    xt_sb[:, (jj * 4 + j) * 128:
                                          (jj * 4 + j + 1) * 128],
                                    start=False, stop=(j == nb - 1),
                                )
                            src = [ps_of[b0 + j] for j in range(nb)]
                        else:
                            agg = wpool.tile([128, 4, 128], fp32, tag="agg",
                                             bufs=3)
                            for j in range(nb):
                                nc.scalar.copy(agg[:, j, :], ps_of[b0 + j])
                            ps2 = p2pool.tile([128, 4, 128], fp32, tag="ps2")
                            for j in range(nb):
                                nc.tensor.matmul(
                                    ps2[:, j, :], w_sb[f"wl{l}"][:],
                                    agg[:, j, :], start=(j == 0), stop=False,
                                )
                                nc.tensor.matmul(
                                    ps2[:, j, :], w_sb[f"wr{l}"][:],
                                    hin[:, jj * 4 + j, :],
                                    start=False, stop=(j == nb - 1),
                                )
                            src = [ps2[:, j, :] for j in range(nb)]
                        if l < 4:
                            for j in range(nb):
                                nc.scalar.activation(
                                    hst[:, jj * 4 + j, :], src[j], func,
                                    bias=w_sb[f"b{l}"][:],
                                )
                            pt = ptpool.tile([128, 4, 128], fp32, tag="pT")
                            for j in range(nb):
                                nc.tensor.transpose(
                                    pt[:, j, :], hst[:, jj * 4 + j, :], ident[:]
                                )
                            nc.vector.tensor_copy(
                                nmst[:, jj * 4:jj * 4 + nb, :], pt[:, :nb, :]
                            )
                        else:
                            for j in range(nb):
                                nc.scalar.activation(
                                    ost[:, jj * 4 + j, :], src[j], func,
                                    bias=w_sb["b4"][:],
                                )
                    if l < 4:
                        nc.sync.dma_start(
                            hfm[l - 1][:, zg * ZROWS:(zg + 1) * ZROWS],
                            hst.rearrange("p a b -> p (a b)"),
                        )
                        nc.sync.dma_start(
                            nm_view(sh[l])[:, zg * GRP:(zg + 1) * GRP, :],
                            nmst[:],
                        )
                    else:
                        nc.sync.dma_start(
                            out_d.ap()[:, zg * ZROWS:(zg + 1) * ZROWS],
                            ost.rearrange("p a b -> p (a b)"),
                        )
                if l < 4:
                    allgather(l)

    nc.compile()
    return nc


def _get_program(CH):
    global _compiled, _compiled_key
    key = CH.tobytes()
    if _compiled is None or _compiled_key != key:
        _compiled = _build_program(CH)
        _compiled_key = key
    return _compiled


# ---------------------------------------------------------------- host side
def make_in_maps(x, edge_index, weights, CH):
    import ml_dtypes

    lay = _layout(CH)
    TOTCH = lay["totch"]
    chunk_base = lay["chunk_base"]

    src = np.asarray(edge_index[0]).astype(np.int64)
    dst = np.asarray(edge_index[1]).astype(np.int64)
    x = np.asarray(x, dtype=np.float32)

    core = dst // SHARD
    dst_loc = dst - core * SHARD
    b = dst_loc // 128
    cs = src // SHARD
    ss = src - cs * SHARD
    g = cs // 2
    row_local = (cs % 2) * SHARD_P + ss

    # position of each edge within its (core, block, range) group
    gid = ((core * NBLK + b) * NR + g).astype(np.int64)
    order = np.lexsort((src, gid))
    gid_s = gid[order]
    cnt = np.bincount(gid_s, minlength=NCORES * NBLK * NR)
    starts = np.concatenate([[0], np.cumsum(cnt)[:-1]])
    pos = np.arange(E, dtype=np.int64) - starts[gid_s]

    slot = (chunk_base[b[order], g[order]] * 128
            + (pos // 128) * 128 + pos % 128)
    # chunk_base is per-core-identical; slot indexes this core's flat array

    TOTSLOT = TOTCH * 128
    gflat = np.zeros((NCORES, TOTSLOT), dtype=np.int16)
    sflat = np.full((NCORES, TOTSLOT), PAD_SLOT, dtype=np.int16)
    core_s = core[order]
    gflat[core_s, slot] = row_local[order].astype(np.int16)
    sflat[core_s, slot] = dst_loc[order].astype(np.int16)

    # wrap: position p -> [p%16, p//16], replicated over 8 groups of 16 parts
    gidx = np.ascontiguousarray(
        np.tile(gflat.reshape(NCORES, -1, 16).transpose(0, 2, 1), (1, 8, 1))
    )

    in_maps = []
    for c in range(NCORES):
        mc = core == c
        cnt_c = np.bincount(dst_loc[mc], minlength=SHARD).astype(np.float32)
        invc = 1.0 / np.maximum(cnt_c, 1.0)

        oh = np.zeros((SHARD_P, H), dtype=np.float32)
        sl = np.arange(SHARD)
        oh[sl, sl % 128] = invc

        # host-built S stream: sdata[p, c*128 + n] = S_flat[c*128 + p, n]
        sflat_c = np.asarray(sflat[c], dtype=np.int64)
        sdata = (
            oh[sflat_c]
            .reshape(TOTCH, 128, H)
            .transpose(1, 0, 2)
            .reshape(128, TOTCH * H)
            .astype(ml_dtypes.bfloat16)
        )

        xt = np.zeros((F_IN, SHARD_P), np.float32)
        xt[:, :SHARD] = x[c * SHARD:(c + 1) * SHARD].T

        im = {
            "xt": xt,
            "gidx": gidx[c],
            "sdata": np.ascontiguousarray(sdata),
        }
        for l in range(1, 5):
            im[f"wl{l}"] = np.asarray(weights[f"Wl{l}"], np.float32)
            im[f"wr{l}"] = np.asarray(weights[f"Wr{l}"], np.float32)
            im[f"b{l}"] = np.asarray(weights[f"b{l}"], np.float32).reshape(128, 1)
        in_maps.append(im)
    return in_maps


def kernel(x, edge_index, Wl1, Wr1, b1, Wl2, Wr2, b2, Wl3, Wr3, b3,
           Wl4, Wr4, b4, _trace=False, _trace_kwargs=None):
    from concourse.bass_utils import run_bass_kernel_spmd

    weights = {
        "Wl1": Wl1, "Wr1": Wr1, "b1": b1,
        "Wl2": Wl2, "Wr2": Wr2, "b2": b2,
        "Wl3": Wl3, "Wr3": Wr3, "b3": b3,
        "Wl4": Wl4, "Wr4": Wr4, "b4": b4,
    }
    CH = _compute_ch(edge_index)
    nc = _get_program(CH)
    in_maps = make_in_maps(x, edge_index, weights, CH)
    res = run_bass_kernel_spmd(
        nc,
        in_maps,
        core_ids=list(range(NCORES)),
        trace=_trace,
        **(_trace_kwargs or {}),
    )
    shards = [res.results[c]["out"].T[:SHARD] for c in range(NCORES)]
    out = np.concatenate(shards, axis=0).astype(np.float32)
    if _trace:
        return out, res
    return out


# revision 3
# speedup vs baseline: 909.4001x; 909.4001x over previous
"""4-layer GraphSAGE (mean aggr) on 8 TRN2 NeuronCores — gather + S-matmul.

Strategy (dst-owner node partitioning, NO scatter-add):
  - Nodes partitioned across 8 cores by dst ownership (12500 each, padded to
    12544 = 98*128).  Feature tables (bf16, node-major) are replicated via
    chunked AllGather (7 chunks of 14 blocks) so AG overlaps compute.
  - Aggregation per 128-node block via TensorE "segment matmul": for each
    128-edge chunk, PSUM[f, n] += G_chunk.T @ S_chunk where G = gathered src
    features [128e, 128f] (bf16) and S[e, n] = invc[dst] * onehot(dst%128)
    (bf16, folds the 1/deg mean normalization).  Pad edges have all-zero S
    rows, so no zeroing passes and no scatter WAW chains exist at all.
  - S is generated once on-device by gathering rows of a small uploaded
    [12544, 128] scaled-onehot table with idx = dst slot, teed to DRAM and
    re-streamed (contiguous reads) for layers 2-4.
  - int16 gather indices: table stored z-chunk-major ([zchunk, core, 1792,
    128]) so each AllGather chunk is contiguous; gathers are split over 4
    row ranges of <=28672 rows so indices fit in int16.
  - Edge slots per (block, range) are padded to a multiple of 128 and to the
    max over the 8 cores (SPMD: all cores run the same program structure,
    computed from the actual graph at first call).
  - Phase C per block: ACT copies PSUM agg -> SBUF, weight matmuls
    (agg @ Wl + h @ Wr, fp32), Relu+bias on ACT, bf16 cast + PE transpose to
    node-major for the next table, fp32 feature-major copy kept in DRAM for
    the next layer's self term.  Layer 1 is transform-first (table1 = x@Wl1)
    and adds Wr1 @ x directly into the agg PSUM.
"""

import numpy as np

# ---------------------------------------------------------------- constants
NCORES = 8
N = 100000
E = 1600000
F_IN = 16
H = 128
SHARD = 12500
NBLK = 98
SHARD_P = NBLK * 128          # 12544
GRP = 14                      # blocks per group == AllGather chunk
NGRP = 7
ZROWS = GRP * 128             # 1792 rows per (core, zchunk)
ZTBL = NCORES * ZROWS         # 14336 table rows per zchunk
TBL_ROWS = NGRP * ZTBL        # 100352
NR = 4                        # gather ranges (int16: 25088 rows each)
RANGE_ROWS = TBL_ROWS // NR   # 25088 = 2 core-shards
MAXC = 8                      # max 128-edge chunks per gather instruction
PAD_SLOT = SHARD_P - 1        # dst slot for pad edges: oh row is all-zero

_compiled = None
_compiled_key = None


# ---------------------------------------------------------------- layout
def _layout(CH):
    """Static program structure from CH[98][4] chunk counts.

    Chunk order: for zg, for g, for b in group(zg): CH[b][g] chunks.
    Instructions split each (zg, g) span into ceil(n/MAXC) near-equal parts.
    Returns dict with chunk_base[98][4], instrs[zg][g] = list of
    (chunk0, nch), chunkinfo[totch] = (block, start, stop), totch.
    """
    CH = np.asarray(CH, dtype=np.int64)
    assert CH.shape == (NBLK, NR) and (CH >= 1).all()
    chunk_base = np.zeros((NBLK, NR), dtype=np.int64)
    instrs = [[[] for _ in range(NR)] for _ in range(NGRP)]
    chunkinfo = []
    tot = 0
    for zg in range(NGRP):
        blocks = range(zg * GRP, (zg + 1) * GRP)
        for g in range(NR):
            span0 = tot
            for b in blocks:
                chunk_base[b, g] = tot
                # PSUM accumulation-group flags are per 4-block PSUM TILE
                # (start clears whole-bank state, so exactly one start/stop
                # per tile): tile jj covers blocks [t0, t1].
                jj = (b - zg * GRP) // 4
                t0 = zg * GRP + jj * 4
                t1 = min(t0 + 3, (zg + 1) * GRP - 1)
                for k in range(CH[b, g]):
                    first = g == 0 and k == 0 and b == t0
                    last = g == NR - 1 and k == CH[b, g] - 1 and b == t1
                    chunkinfo.append((b, first, last))
                tot += CH[b, g]
            n = tot - span0
            nin = -(-n // MAXC)
            per = -(-n // nin)
            c0 = span0
            while c0 < tot:
                nch = min(per, tot - c0)
                instrs[zg][g].append((c0, nch))
                c0 += nch
    return {
        "chunk_base": chunk_base,
        "instrs": instrs,
        "chunkinfo": chunkinfo,
        "totch": tot,
    }


def _compute_ch(edge_index):
    src = np.asarray(edge_index[0]).astype(np.int64)
    dst = np.asarray(edge_index[1]).astype(np.int64)
    core = dst // SHARD
    dst_loc = dst - core * SHARD
    b = dst_loc // 128
    cs = src // SHARD
    ss = src - cs * SHARD
    hh = ss // (SHARD_P // 2)
    g = hh * 2 + cs // 4
    counts = np.zeros((NCORES, NBLK, NR), dtype=np.int64)
    np.add.at(counts, (core, b, g), 1)
    ch = -(-counts.max(axis=0) // 128)
    return np.maximum(ch, 1)


# ---------------------------------------------------------------- program
def _build_program(CH):
    import concourse.bacc as bacc
    import concourse.masks as masks
    import concourse.mybir as mybir
    import concourse.tile as tile

    fp32 = mybir.dt.float32
    bf16 = mybir.dt.bfloat16
    i16 = mybir.dt.int16
    AF = mybir.ActivationFunctionType

    lay = _layout(CH)
    TOTCH = lay["totch"]
    IDXCOLS = TOTCH * 8
    chunkinfo = lay["chunkinfo"]
    instrs = lay["instrs"]

    nc = bacc.Bacc(
        "TRN2",
        target_bir_lowering=False,
        debug=False,
        enable_asserts=False,
        num_devices=NCORES,
    )

    # -------- I/O
    xt_d = nc.dram_tensor("xt", [F_IN, SHARD_P], fp32, kind="ExternalInput")
    gidx_d = nc.dram_tensor("gidx", [128, IDXCOLS], i16, kind="ExternalInput")
    sdata_d = nc.dram_tensor("sdata", [128, TOTCH * 128], bf16,
                             kind="ExternalInput")
    w_d = {}
    for l in range(1, 5):
        din = F_IN if l == 1 else H
        w_d[f"wl{l}"] = nc.dram_tensor(f"wl{l}", [din, H], fp32, kind="ExternalInput")
        w_d[f"wr{l}"] = nc.dram_tensor(f"wr{l}", [din, H], fp32, kind="ExternalInput")
        w_d[f"b{l}"] = nc.dram_tensor(f"b{l}", [128, 1], fp32, kind="ExternalInput")
    out_d = nc.dram_tensor("out", [128, SHARD_P], fp32, kind="ExternalOutput")

    with tile.TileContext(nc) as tc:
        with (
            tc.tile_pool(name="dram", bufs=1, space="DRAM") as dpool,
            tc.tile_pool(name="const", bufs=1) as cpool,
            tc.tile_pool(name="ix", bufs=6) as xpool,
            tc.tile_pool(name="gat", bufs=3) as gpool,
            tc.tile_pool(name="smat", bufs=3) as spool,
            tc.tile_pool(name="hin", bufs=2) as hpool,
            tc.tile_pool(name="xtin", bufs=2) as xtpool,
            tc.tile_pool(name="work", bufs=2) as wpool,
            tc.tile_pool(name="psA", bufs=5, space="PSUM") as papool,
            tc.tile_pool(name="ps2", bufs=2, space="PSUM") as p2pool,
            tc.tile_pool(name="pT", bufs=1, space="PSUM") as ptpool,
        ):
            # two half-tables per layer: the first half's AllGather overlaps
            # the second half's phase C, and next-layer range-0/1 gathers
            # depend only on half 0
            tbl = [
                [dpool.tile([TBL_ROWS // 2, H], bf16, addr_space="Shared",
                            name=f"tbl{l}_h{h}")
                 for h in range(2)]
                for l in range(4)
            ]
            sh = [dpool.tile([SHARD_P, H], bf16, name=f"sh{l}") for l in range(4)]
            hfm = [
                dpool.tile([128, SHARD_P], fp32, name=f"hfm{l}")
                for l in range(1, 4)
            ]

            # -------- constants
            ident = cpool.tile([128, 128], fp32, name="ident")
            masks.make_identity(nc, ident[:])
            w_sb = {}
            for l in range(1, 5):
                din = F_IN if l == 1 else H
                for nm in (f"wl{l}", f"wr{l}"):
                    t = cpool.tile([din, H], fp32, name=f"{nm}_sb")
                    nc.sync.dma_start(t[:], w_d[nm].ap())
                    w_sb[nm] = t
                t = cpool.tile([128, 1], fp32, name=f"b{l}_sb")
                nc.sync.dma_start(t[:], w_d[f"b{l}"].ap())
                w_sb[f"b{l}"] = t

            def nm_view(ap):
                return ap.rearrange("(b p) f -> p b f", p=128)

            def allgather(l, half):
                nc.gpsimd.collective_compute(
                    "AllGather",
                    mybir.AluOpType.bypass,
                    replica_groups=[list(range(NCORES))],
                    ins=[sh[l][half * (SHARD_P // 2):
                               (half + 1) * (SHARD_P // 2), :].opt()],
                    outs=[tbl[l][half][:, :].opt()],
                )

            # ---- phase A: sh0 = bf16(x @ Wl1) node-major; AG -> tbl0
            for zg in range(NGRP):
                xt_sb = xtpool.tile([F_IN, ZROWS], fp32, tag="xtA")
                nc.sync.dma_start(
                    xt_sb[:], xt_d.ap()[:, zg * ZROWS:(zg + 1) * ZROWS]
                )
                nmst = wpool.tile([128, GRP, 128], bf16, tag="nmA")
                for q in range(0, GRP, 4):
                    nb = min(4, GRP - q)
                    ps = p2pool.tile([128, 4, 128], fp32, tag="ps2")
                    psf = ps.rearrange("p a b -> p (a b)")
                    nc.tensor.matmul(
                        psf[:, :nb * 128], w_sb["wl1"][:],
                        xt_sb[:, q * 128:(q + nb) * 128],
                        start=True, stop=True,
                    )
                    csb = wpool.tile([128, 4, 128], fp32, tag="csbA", bufs=2)
                    nc.scalar.copy(
                        csb.rearrange("p a b -> p (a b)")[:, :nb * 128],
                        psf[:, :nb * 128],
                    )
                    pt = ptpool.tile([128, 4, 128], fp32, tag="pT")
                    for j in range(nb):
                        nc.tensor.transpose(pt[:, j, :], csb[:, j, :], ident[:])
                    nc.vector.tensor_copy(nmst[:, q:q + nb, :], pt[:, :nb, :])
                nc.sync.dma_start(
                    nm_view(sh[0])[:, zg * GRP:(zg + 1) * GRP, :], nmst[:]
                )
                if zg == 3:
                    allgather(0, 0)
            allgather(0, 1)

            # ---- layers
            for l in range(1, 5):
                for zg in range(NGRP):
                    # phase B: gathers + segment matmuls
                    ps_of = {}
                    for jj in range(4):
                        nb = min(4, GRP - jj * 4)
                        t = papool.tile([128, 4, 128], fp32, tag="psA")
                        for j in range(nb):
                            ps_of[zg * GRP + jj * 4 + j] = t[:, j, :]
                    for g in range(NR):
                        for (c0, nch) in instrs[zg][g]:
                            nidx = nch * 128
                            ix = xpool.tile([128, MAXC * 8], i16, tag="gix")
                            nc.sync.dma_start(
                                ix[:, :nch * 8],
                                gidx_d.ap()[:, c0 * 8:(c0 + nch) * 8],
                            )
                            gt = gpool.tile([128, MAXC, 128], bf16, tag="gt")
                            nc.gpsimd.dma_gather(
                                gt[:, :nch, :],
                                tbl[l - 1][g // 2][
                                    (g % 2) * RANGE_ROWS:
                                    (g % 2 + 1) * RANGE_ROWS, :],
                                ix[:, :nch * 8],
                                num_idxs=nidx, num_idxs_reg=nidx,
                                elem_size=H,
                            )
                            ss = spool.tile([128, MAXC, 128], bf16, tag="ss")
                            nc.sync.dma_start(
                                ss.rearrange("p a b -> p (a b)")[:, :nch * 128],
                                sdata_d.ap()[:, c0 * 128:(c0 + nch) * 128],
                            )
                            for c in range(nch):
                                b, first, last = chunkinfo[c0 + c]
                                stop = last and l != 1
                                nc.tensor.matmul(
                                    ps_of[b], gt[:, c, :], ss[:, c, :],
                                    start=first, stop=stop,
                                )

                    # phase C
                    if l == 1:
                        xt_sb = xtpool.tile([F_IN, ZROWS], fp32, tag="xtC")
                        nc.sync.dma_start(
                            xt_sb[:], xt_d.ap()[:, zg * ZROWS:(zg + 1) * ZROWS]
                        )
                    elif l >= 2:
                        hin = hpool.tile([128, GRP, 128], fp32, tag="hin")
                        nc.sync.dma_start(
                            hin.rearrange("p a b -> p (a b)"),
                            hfm[l - 2][:, zg * ZROWS:(zg + 1) * ZROWS],
                        )
                    if l < 4:
                        hst = wpool.tile([128, GRP, 128], fp32, tag="hst")
                        nmst = wpool.tile([128, GRP, 128], bf16, tag="nmC")
                    else:
                        ost = wpool.tile([128, GRP, 128], fp32, tag="ost")
                    func = AF.Relu if l < 4 else AF.Identity
                    for jj in range(4):
                        nb = min(4, GRP - jj * 4)
                        b0 = zg * GRP + jj * 4
                        paf = None
                        if l == 1:
                            for j in range(nb):
                                nc.tensor.matmul(
                                    ps_of[b0 + j], w_sb["wr1"][:],
                                    xt_sb[:, (jj * 4 + j) * 128:
                                          (jj * 4 + j + 1) * 128],
                                    start=False, stop=(j == nb - 1),
                                )
                            src = [ps_of[b0 + j] for j in range(nb)]
                        else:
                            agg = wpool.tile([128, 4, 128], fp32, tag="agg",
                                             bufs=3)
                            for j in range(nb):
                                nc.scalar.copy(agg[:, j, :], ps_of[b0 + j])
                            ps2 = p2pool.tile([128, 4, 128], fp32, tag="ps2")
                            for j in range(nb):
                                nc.tensor.matmul(
                                    ps2[:, j, :], w_sb[f"wl{l}"][:],
                                    agg[:, j, :], start=(j == 0), stop=False,
                                )
                                nc.tensor.matmul(
                                    ps2[:, j, :], w_sb[f"wr{l}"][:],
                                    hin[:, jj * 4 + j, :],
                                    start=False, stop=(j == nb - 1),
                                )
                            src = [ps2[:, j, :] for j in range(nb)]
                        if l < 4:
                            for j in range(nb):
                                nc.scalar.activation(
                                    hst[:, jj * 4 + j, :], src[j], func,
                                    bias=w_sb[f"b{l}"][:],
                                )
                            pt = ptpool.tile([128, 4, 128], fp32, tag="pT")
                            for j in range(nb):
                                nc.tensor.transpose(
                                    pt[:, j, :], hst[:, jj * 4 + j, :], ident[:]
                                )
                            nc.vector.tensor_copy(
                                nmst[:, jj * 4:jj * 4 + nb, :], pt[:, :nb, :]
                            )
                        else:
                            for j in range(nb):
                                nc.scalar.activation(
                                    ost[:, jj * 4 + j, :], src[j], func,
                                    bias=w_sb["b4"][:],
                                )
                    if l < 4:
                        nc.sync.dma_start(
                            hfm[l - 1][:, zg * ZROWS:(zg + 1) * ZROWS],
                            hst.rearrange("p a b -> p (a b)"),
                        )
                        nc.sync.dma_start(
                            nm_view(sh[l])[:, zg * GRP:(zg + 1) * GRP, :],
                            nmst[:],
                        )
                        if zg == 3:
                            allgather(l, 0)
                        elif zg == NGRP - 1:
                            allgather(l, 1)
                    else:
                        nc.sync.dma_start(
                            out_d.ap()[:, zg * ZROWS:(zg + 1) * ZROWS],
                            ost.rearrange("p a b -> p (a b)"),
                        )
    nc.compile()
    return nc


def _get_program(CH):
    global _compiled, _compiled_key
    key = CH.tobytes()
    if _compiled is None or _compiled_key != key:
        _compiled = _build_program(CH)
        _compiled_key = key
    return _compiled


# ---------------------------------------------------------------- host side
def make_in_maps(x, edge_index, weights, CH):
    import ml_dtypes

    lay = _layout(CH)
    TOTCH = lay["totch"]
    chunk_base = lay["chunk_base"]

    src = np.asarray(edge_index[0]).astype(np.int64)
    dst = np.asarray(edge_index[1]).astype(np.int64)
    x = np.asarray(x, dtype=np.float32)

    core = dst // SHARD
    dst_loc = dst - core * SHARD
    b = dst_loc // 128
    cs = src // SHARD
    ss = src - cs * SHARD
    hh = ss // (SHARD_P // 2)
    g = hh * 2 + cs // 4
    row_local = (cs % 4) * (SHARD_P // 2) + (ss - hh * (SHARD_P // 2))

    # position of each edge within its (core, block, range) group
    gid = ((core * NBLK + b) * NR + g).astype(np.int64)
    order = np.lexsort((src, gid))
    gid_s = gid[order]
    cnt = np.bincount(gid_s, minlength=NCORES * NBLK * NR)
    starts = np.concatenate([[0], np.cumsum(cnt)[:-1]])
    pos = np.arange(E, dtype=np.int64) - starts[gid_s]

    slot = (chunk_base[b[order], g[order]] * 128
            + (pos // 128) * 128 + pos % 128)
    # chunk_base is per-core-identical; slot indexes this core's flat array

    TOTSLOT = TOTCH * 128
    gflat = np.zeros((NCORES, TOTSLOT), dtype=np.int16)
    sflat = np.full((NCORES, TOTSLOT), PAD_SLOT, dtype=np.int16)
    core_s = core[order]
    gflat[core_s, slot] = row_local[order].astype(np.int16)
    sflat[core_s, slot] = dst_loc[order].astype(np.int16)

    # wrap: position p -> [p%16, p//16], replicated over 8 groups of 16 parts
    gidx = np.ascontiguousarray(
        np.tile(gflat.reshape(NCORES, -1, 16).transpose(0, 2, 1), (1, 8, 1))
    )

    in_maps = []
    for c in range(NCORES):
        mc = core == c
        cnt_c = np.bincount(dst_loc[mc], minlength=SHARD).astype(np.float32)
        invc = 1.0 / np.maximum(cnt_c, 1.0)

        oh = np.zeros((SHARD_P, H), dtype=np.float32)
        sl = np.arange(SHARD)
        oh[sl, sl % 128] = invc

        # host-built S stream: sdata[p, c*128 + n] = S_flat[c*128 + p, n]
        sflat_c = np.asarray(sflat[c], dtype=np.int64)
        sdata = (
            oh[sflat_c]
            .reshape(TOTCH, 128, H)
            .transpose(1, 0, 2)
            .reshape(128, TOTCH * H)
            .astype(ml_dtypes.bfloat16)
        )

        xt = np.zeros((F_IN, SHARD_P), np.float32)
        xt[:, :SHARD] = x[c * SHARD:(c + 1) * SHARD].T

        im = {
            "xt": xt,
            "gidx": gidx[c],
            "sdata": np.ascontiguousarray(sdata),
        }
        for l in range(1, 5):
            im[f"wl{l}"] = np.asarray(weights[f"Wl{l}"], np.float32)
            im[f"wr{l}"] = np.asarray(weights[f"Wr{l}"], np.float32)
            im[f"b{l}"] = np.asarray(weights[f"b{l}"], np.float32).reshape(128, 1)
        in_maps.append(im)
    return in_maps


def kernel(x, edge_index, Wl1, Wr1, b1, Wl2, Wr2, b2, Wl3, Wr3, b3,
           Wl4, Wr4, b4, _trace=False, _trace_kwargs=None):
    from concourse.bass_utils import run_bass_kernel_spmd

    weights = {
        "Wl1": Wl1, "Wr1": Wr1, "b1": b1,
        "Wl2": Wl2, "Wr2": Wr2, "b2": b2,
        "Wl3": Wl3, "Wr3": Wr3, "b3": b3,
        "Wl4": Wl4, "Wr4": Wr4, "b4": b4,
    }
    CH = _compute_ch(edge_index)
    nc = _get_program(CH)
    in_maps = make_in_maps(x, edge_index, weights, CH)
    res = run_bass_kernel_spmd(
        nc,
        in_maps,
        core_ids=list(range(NCORES)),
        trace=_trace,
        **(_trace_kwargs or {}),
    )
    shards = [res.results[c]["out"].T[:SHARD] for c in range(NCORES)]
    out = np.concatenate(shards, axis=0).astype(np.float32)
    if _trace:
        return out, res
    return out
